# revision 19
# baseline (speedup 1.0000x reference)
"""Causal multi-head attention (B=4, T=2048, C=1024, H=16) on 8 TRN2 NeuronCores.

Sharding: core c handles batch b=c//2 and head-half r=c%2 (8 of 16 heads).
Every core runs an IDENTICAL graph (full causal attention for its 8 heads over
all T tokens) -> pure SPMD, no collectives. The output projection is
row-parallel over the head-halves; the host sums the two partial z's per batch
(the unshard step) and adds the bias-fold vector Wp@bv.

Device layout choices:
  - everything enters as bf16 (host pre-casts); matmuls accumulate fp32 in PSUM
  - qT/kT stored [d, t] with head pairs stacked 64+64 on partitions ->
    K=64 row-tiled matmul pairs use both halves of the PE array concurrently
  - scores computed transposed S^T=[k, q]; exp on ScalarE (scale=1/8 fused);
    causal handled by N-trimming each matmul + one 128x128 triangle mask mul
  - softmax denominator l = sum_k exp computed for free by an all-ones column
    appended to v (fp32 PSUM accumulation); y^T = v_aug^T @ P^T
  - 1/l broadcast across partitions via DVE reciprocal + gpsimd
    partition_broadcast, then one DVE multiply normalizes
"""

import os
from contextlib import ExitStack

import numpy as np
import ml_dtypes

import concourse.tile as tile
from concourse import bacc, mybir


def _ensure_axon_hooks():
    """bass_utils' axon trace path does a hard import of antenv.axon_hooks,
    which this image's antenv lacks. Provide the module (with a real NTFF
    hook when the axon .so supports it) so trace=True / BASS_TRACE=1 works;
    harmless when tracing is off."""
    try:
        import antenv.axon_hooks  # noqa: F401
        return
    except ImportError:
        pass
    import sys
    import types
    try:
        import antenv
    except ImportError:
        return
    m = types.ModuleType("antenv.axon_hooks")
    m._hook = None

    def set_axon_ntff_profile_hook(h):
        m._hook = h

    def get_axon_ntff_profile_hook():
        return m._hook

    m.set_axon_ntff_profile_hook = set_axon_ntff_profile_hook
    m.get_axon_ntff_profile_hook = get_axon_ntff_profile_hook
    sys.modules["antenv.axon_hooks"] = m
    antenv.axon_hooks = m


_ensure_axon_hooks()

from concourse.bass_utils import run_bass_kernel_spmd  # noqa: E402

BF = ml_dtypes.bfloat16
B, T, C, H, HD = 4, 2048, 1024, 16, 64
NCORES = 8
DH = C // 2        # 512 d-dims per core (8 heads)
NPAIR = 4          # head pairs per core
NQB = T // 512     # 4 query blocks of 512
NKB = T // 128     # 16 key/token blocks of 128
NCH = C // 128     # 8 contraction chunks
f32 = mybir.dt.float32
bf16 = mybir.dt.bfloat16

_CACHED_NC = None
LAST_RESULTS = None  # BassKernelResults of the most recent run


def _build_nc():
    nc = bacc.Bacc("TRN2", target_bir_lowering=False, debug=False,
                   num_devices=NCORES)
    AF = mybir.ActivationFunctionType

    xT = nc.dram_tensor("xT", [C, T], bf16, kind="ExternalInput").ap()
    wqT = nc.dram_tensor("wqT", [C, DH], bf16, kind="ExternalInput").ap()
    wkT = nc.dram_tensor("wkT", [C, DH], bf16, kind="ExternalInput").ap()
    wvT = nc.dram_tensor("wvT", [C, DH], bf16, kind="ExternalInput").ap()
    wpT = nc.dram_tensor("wpT", [DH, C], bf16, kind="ExternalInput").ap()
    bqD = nc.dram_tensor("bq", [NPAIR, 128, 1], f32, kind="ExternalInput").ap()
    bkD = nc.dram_tensor("bk", [NPAIR, 128, 1], f32, kind="ExternalInput").ap()
    triD = nc.dram_tensor("tri", [128, 128], bf16, kind="ExternalInput").ap()
    zD = nc.dram_tensor("z", [T, C], mybir.dt.float16,
                    kind="ExternalOutput").ap()

    with tile.TileContext(nc) as tc, ExitStack() as ctx:
        const = ctx.enter_context(tc.tile_pool(name="const", bufs=1))
        qkp = ctx.enter_context(tc.tile_pool(name="qk", bufs=1))
        vp = ctx.enter_context(tc.tile_pool(name="vp", bufs=1))
        ynp = ctx.enter_context(tc.tile_pool(name="yn", bufs=1))
        ptp = ctx.enter_context(tc.tile_pool(name="pt", bufs=5))
        smallp = ctx.enter_context(tc.tile_pool(name="small", bufs=2))
        bcp = ctx.enter_context(tc.tile_pool(name="bc", bufs=2))
        zstp = ctx.enter_context(tc.tile_pool(name="zst", bufs=3))
        yevp = ctx.enter_context(tc.tile_pool(name="yev", bufs=4))
        stgp = ctx.enter_context(tc.tile_pool(name="stg", bufs=2))
        ps = ctx.enter_context(tc.tile_pool(name="ps", bufs=2, space="PSUM"))
        ps2 = ctx.enter_context(tc.tile_pool(name="ps2", bufs=2, space="PSUM"))
        yps = ctx.enter_context(tc.tile_pool(name="yps", bufs=2, space="PSUM"))

        # ---- tiny constants first: they gate pair-0's bias-adds/masks
        tri2 = const.tile([128, 2, 128], bf16, tag="tri2")
        nc.sync.dma_start(out=tri2[:, 0, :], in_=triD[:, :])
        nc.sync.dma_start(out=tri2[:, 1, :], in_=triD[:, :])
        bq_sb, bk_sb = [], []
        for hp in range(NPAIR):
            tq = const.tile([128, 1], f32, tag=f"bq{hp}", name=f"bq{hp}")
            nc.sync.dma_start(out=tq[:, :], in_=bqD[hp, :, :])
            bq_sb.append(tq)
            tk = const.tile([128, 1], f32, tag=f"bk{hp}", name=f"bk{hp}")
            nc.sync.dma_start(out=tk[:, :], in_=bkD[hp, :, :])
            bk_sb.append(tk)

        # ---- resident inputs (interleaved by c-chunk so the first
        # projection matmul can start after ~2 tiles land)
        xT_sb, wq_sb, wk_sb, wv_sb = [], [], [], []
        for j in range(NCH):
            for lst, src_, ncols, tagp in ((xT_sb, xT, T, "xT"),
                                           (wq_sb, wqT, DH, "wq"),
                                           (wk_sb, wkT, DH, "wk"),
                                           (wv_sb, wvT, DH, "wv")):
                t_ = const.tile([128, ncols], bf16, tag=f"{tagp}{j}",
                                name=f"{tagp}{j}")
                nc.sync.dma_start(out=t_[:, :],
                                  in_=src_[j * 128:(j + 1) * 128, :])
                lst.append(t_)
        wp_sb = []
        for j in range(NPAIR):
            t_ = const.tile([128, C], bf16, tag=f"wp{j}", name=f"wp{j}")
            nc.sync.dma_start(out=t_[:, :], in_=wpT[j * 128:(j + 1) * 128, :])
            wp_sb.append(t_)
        # ---- V projection: one tile holds v for all 8 local heads,
        # [k%128, kblock, head, 64 dims + ones column + zero pad to 128
        # (full 128-col weights enable PE fast-weight-load)]
        vt = vp.tile([128, NKB, 8, 128], bf16, tag="vt", name="vt")
        nc.vector.memset(vt[:, :, :, 64:65], 1.0)
        nc.vector.memset(vt[:, :, :, 65:128], 0.0)

        def emit_vproj(i0, i1):
            for i in range(i0, i1):
                p_ = ps.tile([128, DH], f32, tag="ps", name=f"vps{i}")
                for cj in range(NCH):
                    nc.tensor.matmul(p_[:, :],
                                     lhsT=xT_sb[cj][:, i * 128:(i + 1) * 128],
                                     rhs=wv_sb[cj][:, :],
                                     start=(cj == 0), stop=(cj == NCH - 1))
                nc.vector.tensor_copy(
                    vt[:, i, :, 0:64],
                    p_[:, :].rearrange("p (h e) -> p h e", h=8))

        yn = [ynp.tile([128, T], bf16, tag=f"yn{hp}", name=f"yn{hp}")
              for hp in range(NPAIR)]
        qts, kts = {}, {}

        def emit_qkproj(hp):
            qt = qkp.tile([128, T], bf16, tag=f"qT{hp}", name=f"qT{hp}")
            kt = qkp.tile([128, T], bf16, tag=f"kT{hp}", name=f"kT{hp}")
            qts[hp], kts[hp] = qt, kt
            for j in range(NQB):
                pq = ps.tile([128, 512], f32, tag="ps", name=f"pq{hp}_{j}")
                for cj in range(NCH):
                    nc.tensor.matmul(
                        pq[:, :],
                        lhsT=wq_sb[cj][:, hp * 128:(hp + 1) * 128],
                        rhs=xT_sb[cj][:, j * 512:(j + 1) * 512],
                        start=(cj == 0), stop=(cj == NCH - 1))
                nc.vector.tensor_scalar_add(qt[:, j * 512:(j + 1) * 512],
                                            pq[:, :], bq_sb[hp][:, :])
                pk = ps.tile([128, 512], f32, tag="ps", name=f"pk{hp}_{j}")
                for cj in range(NCH):
                    nc.tensor.matmul(
                        pk[:, :],
                        lhsT=wk_sb[cj][:, hp * 128:(hp + 1) * 128],
                        rhs=xT_sb[cj][:, j * 512:(j + 1) * 512],
                        start=(cj == 0), stop=(cj == NCH - 1))
                nc.vector.tensor_scalar_add(kt[:, j * 512:(j + 1) * 512],
                                            pk[:, :], bk_sb[hp][:, :])

        def emit_attention(hp, Qi):
                qt, kt = qts[hp], kts[hp]
                kmax = 4 * (Qi + 1)
                ya = yps.tile([128, 512], f32, tag="yps")
                yb = yps.tile([128, 512], f32, tag="yps")
                for ch in range(Qi + 1):
                    pts = []
                    for kb in range(4 * ch, 4 * ch + 4):
                        s = max(0, (kb - 4 * Qi) * 128)
                        sAB = ps2.tile([128, 2, 512], f32, tag="ps2")
                        nc.tensor.matmul(
                            sAB[:, 0, s:512],
                            lhsT=kt[0:64, kb * 128:(kb + 1) * 128],
                            rhs=qt[0:64, Qi * 512 + s:(Qi + 1) * 512],
                            start=True, stop=True)
                        nc.tensor.matmul(
                            sAB[:, 1, s:512],
                            lhsT=kt[64:128, kb * 128:(kb + 1) * 128],
                            rhs=qt[64:128, Qi * 512 + s:(Qi + 1) * 512],
                            start=True, stop=True)
                        pt_ = ptp.tile([128, 2, 512], bf16, tag="pt")
                        nc.scalar.activation(pt_[:, :, s:512],
                                             sAB[:, :, s:512],
                                             AF.Exp, scale=0.125)
                        if kb >= 4 * Qi:
                            nc.vector.tensor_mul(pt_[:, :, s:s + 128],
                                                 pt_[:, :, s:s + 128],
                                                 tri2[:, :, :])
                        pts.append((kb, s, pt_))
                    for kb, s, pt_ in pts:
                        nc.tensor.matmul(ya[:, s:512],
                                         lhsT=vt[:, kb, 2 * hp, :],
                                         rhs=pt_[:, 0, s:512],
                                         start=(kb == 0), stop=(kb == kmax - 1))
                    for kb, s, pt_ in pts:
                        nc.tensor.matmul(yb[:, s:512],
                                         lhsT=vt[:, kb, 2 * hp + 1, :],
                                         rhs=pt_[:, 1, s:512],
                                         start=(kb == 0), stop=(kb == kmax - 1))
                # evacuate PSUM immediately, then a per-(pair,Qi) batched
                # reciprocal: l rows DMA-reshaped [1,512]->[128,4] so the
                # DVE reciprocal (~8 cyc/elem PER LANE) sees 8 elems/lane
                yevs = []
                for h, yy in ((0, ya), (1, yb)):
                    yev = yevp.tile([65, 512], f32, tag="yev")
                    nc.vector.tensor_copy(yev[:, :], yy[0:65, :])
                    yevs.append(yev)
                stg = stgp.tile([128, 8], f32, tag="stg")
                rstg = stgp.tile([128, 8], f32, tag="rstg")
                for h in (0, 1):
                    nc.sync.dma_start(out=stg[:, h * 4:(h + 1) * 4],
                                      in_=yevs[h][64:65, :])
                nc.vector.reciprocal(rstg[:, :], stg[:, :])
                for h in (0, 1):
                    rr = smallp.tile([1, 512], f32, tag="rr")
                    nc.sync.dma_start(out=rr[0:1, :],
                                      in_=rstg[:, h * 4:(h + 1) * 4])
                    bc = bcp.tile([64, 512], f32, tag="bc")
                    nc.gpsimd.partition_broadcast(bc[:, :], rr[0:1, :])
                    nc.vector.tensor_mul(
                        yn[hp][h * 64:(h + 1) * 64, Qi * 512:(Qi + 1) * 512],
                        yevs[h][0:64, :], bc[:, :])

        def emit_z(Qi):
            for i in range(4 * Qi, 4 * Qi + 4):
                for j2 in range(2):
                    pz = ps.tile([128, 512], f32, tag="ps", name=f"pz{i}_{j2}")
                    for hp in range(NPAIR):
                        nc.tensor.matmul(
                            pz[:, :],
                            lhsT=yn[hp][:, i * 128:(i + 1) * 128],
                            rhs=wp_sb[hp][:, j2 * 512:(j2 + 1) * 512],
                            start=(hp == 0), stop=(hp == NPAIR - 1))
                    zs = zstp.tile([128, 512], mybir.dt.float16, tag="zst")
                    if (i * 2 + j2) % 2 == 0:
                        nc.vector.tensor_copy(zs[:, :], pz[:, :])
                    else:
                        nc.scalar.copy(zs[:, :], pz[:, :])
                    nc.sync.dma_start(
                        out=zD[i * 128:(i + 1) * 128,
                               j2 * 512:(j2 + 1) * 512],
                        in_=zs[:, :])

        # ---- schedule: pair 0's attention interleaves with the
        # v-projection so ScalarE's exp stream starts early; pair 3 walks
        # its query blocks in descending order with the matching z blocks
        # emitted right after, so the output projection chases pair 3.
        emit_qkproj(0)
        emit_vproj(0, 4)
        emit_attention(0, 0)
        emit_vproj(4, 8)
        emit_attention(0, 1)
        emit_vproj(8, 12)
        emit_attention(0, 2)
        emit_vproj(12, 16)
        emit_attention(0, 3)
        for hp in (1, 2):
            emit_qkproj(hp)
            for Qi in range(NQB):
                emit_attention(hp, Qi)
        emit_qkproj(3)
        for Qi in range(NQB - 1, -1, -1):
            emit_attention(3, Qi)
            emit_z(Qi)

    nc.compile()
    return nc


def get_nc():
    global _CACHED_NC
    if _CACHED_NC is None:
        _CACHED_NC = _build_nc()
    return _CACHED_NC


def make_in_map(core, x, Wq, bq, Wk, bk, Wv, Wp):
    """Host-side shard/layout prep for one core (pure numpy, no FLOP-bearing
    compute: transposes, slicing, dtype casts)."""
    b, r = core // 2, core % 2
    hsl = slice(r * DH, (r + 1) * DH)
    return {
        "xT": np.ascontiguousarray(x[b].T).astype(BF),
        "wqT": np.ascontiguousarray(Wq[hsl, :].T).astype(BF),
        "wkT": np.ascontiguousarray(Wk[hsl, :].T).astype(BF),
        "wvT": np.ascontiguousarray(Wv[hsl, :].T).astype(BF),
        "wpT": np.ascontiguousarray(Wp[:, hsl].T).astype(BF),
        "bq": np.ascontiguousarray(bq[hsl]).reshape(NPAIR, 128, 1)
                .astype(np.float32),
        "bk": np.ascontiguousarray(bk[hsl]).reshape(NPAIR, 128, 1)
                .astype(np.float32),
        "tri": np.triu(np.ones((128, 128), np.float32)).astype(BF),
    }


def kernel(x, Wq, bq, Wk, bk, Wv, bv, Wp):
    global LAST_RESULTS
    x = np.asarray(x, np.float32)
    Wq, bq = np.asarray(Wq, np.float32), np.asarray(bq, np.float32)
    Wk, bk = np.asarray(Wk, np.float32), np.asarray(bk, np.float32)
    Wv, bv = np.asarray(Wv, np.float32), np.asarray(bv, np.float32)
    Wp = np.asarray(Wp, np.float32)

    nc = get_nc()
    in_maps = [make_in_map(c, x, Wq, bq, Wk, bk, Wv, Wp)
               for c in range(NCORES)]
    res = run_bass_kernel_spmd(nc, in_maps, core_ids=list(range(NCORES)))
    LAST_RESULTS = res

    # unshard: sum the two head-half partials per batch; add folded V-bias
    # term (y gets +bv per token; through the output projection that is the
    # constant vector Wp @ bv added to every token)
    zbias = (Wp @ bv).astype(np.float32)
    out = np.empty((B, T, C), np.float32)
    for b in range(B):
        za = np.asarray(res.results[2 * b]["z"], np.float32)
        zb = np.asarray(res.results[2 * b + 1]["z"], np.float32)
        out[b] = za + zb + zbias[None, :]
    return out


# revision 20
# speedup vs baseline: 1.0187x; 1.0187x over previous
"""Causal multi-head attention (B=4, T=2048, C=1024, H=16) on 8 TRN2 NeuronCores.

Sharding: core c handles batch b=c//2 and head-half r=c%2 (8 of 16 heads).
Every core runs an IDENTICAL graph (full causal attention for its 8 heads over
all T tokens) -> pure SPMD, no collectives. The output projection is
row-parallel over the head-halves; the host sums the two partial z's per batch
(the unshard step) and adds the bias-fold vector Wp@bv.

Device layout choices:
  - everything enters as bf16 (host pre-casts); matmuls accumulate fp32 in PSUM
  - qT/kT stored [d, t] with head pairs stacked 64+64 on partitions ->
    K=64 row-tiled matmul pairs use both halves of the PE array concurrently
  - scores computed transposed S^T=[k, q]; exp on ScalarE (scale=1/8 fused);
    causal handled by N-trimming each matmul + one 128x128 triangle mask mul
  - softmax denominator l = sum_k exp computed for free by an all-ones column
    appended to v (fp32 PSUM accumulation); y^T = v_aug^T @ P^T
  - 1/l broadcast across partitions via DVE reciprocal + gpsimd
    partition_broadcast, then one DVE multiply normalizes
"""

import os
from contextlib import ExitStack

import numpy as np
import ml_dtypes

import concourse.tile as tile
from concourse import bacc, mybir


def _ensure_axon_hooks():
    """bass_utils' axon trace path does a hard import of antenv.axon_hooks,
    which this image's antenv lacks. Provide the module (with a real NTFF
    hook when the axon .so supports it) so trace=True / BASS_TRACE=1 works;
    harmless when tracing is off."""
    try:
        import antenv.axon_hooks  # noqa: F401
        return
    except ImportError:
        pass
    import sys
    import types
    try:
        import antenv
    except ImportError:
        return
    m = types.ModuleType("antenv.axon_hooks")
    m._hook = None

    def set_axon_ntff_profile_hook(h):
        m._hook = h

    def get_axon_ntff_profile_hook():
        return m._hook

    m.set_axon_ntff_profile_hook = set_axon_ntff_profile_hook
    m.get_axon_ntff_profile_hook = get_axon_ntff_profile_hook
    sys.modules["antenv.axon_hooks"] = m
    antenv.axon_hooks = m


_ensure_axon_hooks()

from concourse.bass_utils import run_bass_kernel_spmd  # noqa: E402

BF = ml_dtypes.bfloat16
B, T, C, H, HD = 4, 2048, 1024, 16, 64
NCORES = 8
DH = C // 2        # 512 d-dims per core (8 heads)
NPAIR = 4          # head pairs per core
NQB = T // 512     # 4 query blocks of 512
NKB = T // 128     # 16 key/token blocks of 128
NCH = C // 128     # 8 contraction chunks
f32 = mybir.dt.float32
bf16 = mybir.dt.bfloat16

_CACHED_NC = None
LAST_RESULTS = None  # BassKernelResults of the most recent run


def _build_nc():
    nc = bacc.Bacc("TRN2", target_bir_lowering=False, debug=False,
                   num_devices=NCORES)
    AF = mybir.ActivationFunctionType

    xT = nc.dram_tensor("xT", [C, T], bf16, kind="ExternalInput").ap()
    wqT = nc.dram_tensor("wqT", [C, DH], bf16, kind="ExternalInput").ap()
    wkT = nc.dram_tensor("wkT", [C, DH], bf16, kind="ExternalInput").ap()
    wvT = nc.dram_tensor("wvT", [C, DH], bf16, kind="ExternalInput").ap()
    wpT = nc.dram_tensor("wpT", [DH, C], bf16, kind="ExternalInput").ap()
    bqD = nc.dram_tensor("bq", [NPAIR, 128, 1], f32, kind="ExternalInput").ap()
    bkD = nc.dram_tensor("bk", [NPAIR, 128, 1], f32, kind="ExternalInput").ap()
    triD = nc.dram_tensor("tri", [128, 128], bf16, kind="ExternalInput").ap()
    zD = nc.dram_tensor("z", [T, C], mybir.dt.float16,
                    kind="ExternalOutput").ap()

    with tile.TileContext(nc) as tc, ExitStack() as ctx:
        const = ctx.enter_context(tc.tile_pool(name="const", bufs=1))
        qkp = ctx.enter_context(tc.tile_pool(name="qk", bufs=1))
        vp = ctx.enter_context(tc.tile_pool(name="vp", bufs=1))
        ynp = ctx.enter_context(tc.tile_pool(name="yn", bufs=1))
        ptp = ctx.enter_context(tc.tile_pool(name="pt", bufs=5))
        smallp = ctx.enter_context(tc.tile_pool(name="small", bufs=2))
        bcp = ctx.enter_context(tc.tile_pool(name="bc", bufs=2))
        zstp = ctx.enter_context(tc.tile_pool(name="zst", bufs=3))
        yevp = ctx.enter_context(tc.tile_pool(name="yev", bufs=4))
        stgp = ctx.enter_context(tc.tile_pool(name="stg", bufs=2))
        ps = ctx.enter_context(tc.tile_pool(name="ps", bufs=2, space="PSUM"))
        ps2 = ctx.enter_context(tc.tile_pool(name="ps2", bufs=2, space="PSUM"))
        yps = ctx.enter_context(tc.tile_pool(name="yps", bufs=2, space="PSUM"))

        # ---- tiny constants first: they gate pair-0's bias-adds/masks
        tri2 = const.tile([128, 2, 128], bf16, tag="tri2")
        nc.sync.dma_start(out=tri2[:, 0, :], in_=triD[:, :])
        nc.sync.dma_start(out=tri2[:, 1, :], in_=triD[:, :])
        bq_sb, bk_sb = [], []
        for hp in range(NPAIR):
            tq = const.tile([128, 1], f32, tag=f"bq{hp}", name=f"bq{hp}")
            nc.sync.dma_start(out=tq[:, :], in_=bqD[hp, :, :])
            bq_sb.append(tq)
            tk = const.tile([128, 1], f32, tag=f"bk{hp}", name=f"bk{hp}")
            nc.sync.dma_start(out=tk[:, :], in_=bkD[hp, :, :])
            bk_sb.append(tk)

        # ---- resident inputs (interleaved by c-chunk so the first
        # projection matmul can start after ~2 tiles land)
        xT_sb, wq_sb, wk_sb, wv_sb = [], [], [], []
        for j in range(NCH):
            xt_ = const.tile([128, T], bf16, tag=f"xT{j}", name=f"xT{j}")
            nc.sync.dma_start(out=xt_[:, 0:1024],
                              in_=xT[j * 128:(j + 1) * 128, 0:1024])
            xT_sb.append(xt_)
            for lst, src_, tagp in ((wq_sb, wqT, "wq"), (wk_sb, wkT, "wk")):
                t_ = const.tile([128, DH], bf16, tag=f"{tagp}{j}",
                                name=f"{tagp}{j}")
                nc.sync.dma_start(out=t_[:, :],
                                  in_=src_[j * 128:(j + 1) * 128, :])
                lst.append(t_)
        for j in range(NCH):
            t_ = const.tile([128, DH], bf16, tag=f"wv{j}", name=f"wv{j}")
            nc.sync.dma_start(out=t_[:, :], in_=wvT[j * 128:(j + 1) * 128, :])
            wv_sb.append(t_)
            nc.sync.dma_start(out=xT_sb[j][:, 1024:2048],
                              in_=xT[j * 128:(j + 1) * 128, 1024:2048])
        wp_sb = []
        for j in range(NPAIR):
            t_ = const.tile([128, C], bf16, tag=f"wp{j}", name=f"wp{j}")
            nc.sync.dma_start(out=t_[:, :], in_=wpT[j * 128:(j + 1) * 128, :])
            wp_sb.append(t_)
        # ---- V projection: one tile holds v for all 8 local heads,
        # [k%128, kblock, head, 64 dims + ones column + zero pad to 128
        # (full 128-col weights enable PE fast-weight-load)]
        vt = vp.tile([128, NKB, 8, 128], bf16, tag="vt", name="vt")
        nc.vector.memset(vt[:, :, :, 64:65], 1.0)
        nc.vector.memset(vt[:, :, :, 65:128], 0.0)

        def emit_vproj(i0, i1):
            for i in range(i0, i1):
                p_ = ps.tile([128, DH], f32, tag="ps", name=f"vps{i}")
                for cj in range(NCH):
                    nc.tensor.matmul(p_[:, :],
                                     lhsT=xT_sb[cj][:, i * 128:(i + 1) * 128],
                                     rhs=wv_sb[cj][:, :],
                                     start=(cj == 0), stop=(cj == NCH - 1))
                nc.vector.tensor_copy(
                    vt[:, i, :, 0:64],
                    p_[:, :].rearrange("p (h e) -> p h e", h=8))

        yn = [ynp.tile([128, T], bf16, tag=f"yn{hp}", name=f"yn{hp}")
              for hp in range(NPAIR)]
        qts, kts = {}, {}

        def emit_qkproj_part(hp, j):
            if hp not in qts:
                qts[hp] = qkp.tile([128, T], bf16, tag=f"qT{hp}",
                                   name=f"qT{hp}")
                kts[hp] = qkp.tile([128, T], bf16, tag=f"kT{hp}",
                                   name=f"kT{hp}")
            qt, kt = qts[hp], kts[hp]
            if True:
                pq = ps.tile([128, 512], f32, tag="ps", name=f"pq{hp}_{j}")
                for cj in range(NCH):
                    nc.tensor.matmul(
                        pq[:, :],
                        lhsT=wq_sb[cj][:, hp * 128:(hp + 1) * 128],
                        rhs=xT_sb[cj][:, j * 512:(j + 1) * 512],
                        start=(cj == 0), stop=(cj == NCH - 1))
                nc.vector.tensor_scalar_add(qt[:, j * 512:(j + 1) * 512],
                                            pq[:, :], bq_sb[hp][:, :])
                pk = ps.tile([128, 512], f32, tag="ps", name=f"pk{hp}_{j}")
                for cj in range(NCH):
                    nc.tensor.matmul(
                        pk[:, :],
                        lhsT=wk_sb[cj][:, hp * 128:(hp + 1) * 128],
                        rhs=xT_sb[cj][:, j * 512:(j + 1) * 512],
                        start=(cj == 0), stop=(cj == NCH - 1))
                nc.vector.tensor_scalar_add(kt[:, j * 512:(j + 1) * 512],
                                            pk[:, :], bk_sb[hp][:, :])

        def emit_attention(hp, Qi):
                qt, kt = qts[hp], kts[hp]
                kmax = 4 * (Qi + 1)
                ya = yps.tile([128, 512], f32, tag="yps")
                yb = yps.tile([128, 512], f32, tag="yps")
                for ch in range(Qi + 1):
                    pts = []
                    for kb in range(4 * ch, 4 * ch + 4):
                        s = max(0, (kb - 4 * Qi) * 128)
                        sAB = ps2.tile([128, 2, 512], f32, tag="ps2")
                        nc.tensor.matmul(
                            sAB[:, 0, s:512],
                            lhsT=kt[0:64, kb * 128:(kb + 1) * 128],
                            rhs=qt[0:64, Qi * 512 + s:(Qi + 1) * 512],
                            start=True, stop=True)
                        nc.tensor.matmul(
                            sAB[:, 1, s:512],
                            lhsT=kt[64:128, kb * 128:(kb + 1) * 128],
                            rhs=qt[64:128, Qi * 512 + s:(Qi + 1) * 512],
                            start=True, stop=True)
                        pt_ = ptp.tile([128, 2, 512], bf16, tag="pt")
                        nc.scalar.activation(pt_[:, :, s:512],
                                             sAB[:, :, s:512],
                                             AF.Exp, scale=0.125)
                        if kb >= 4 * Qi:
                            nc.vector.tensor_mul(pt_[:, :, s:s + 128],
                                                 pt_[:, :, s:s + 128],
                                                 tri2[:, :, :])
                        pts.append((kb, s, pt_))
                    for kb, s, pt_ in pts:
                        nc.tensor.matmul(ya[:, s:512],
                                         lhsT=vt[:, kb, 2 * hp, :],
                                         rhs=pt_[:, 0, s:512],
                                         start=(kb == 0), stop=(kb == kmax - 1))
                    for kb, s, pt_ in pts:
                        nc.tensor.matmul(yb[:, s:512],
                                         lhsT=vt[:, kb, 2 * hp + 1, :],
                                         rhs=pt_[:, 1, s:512],
                                         start=(kb == 0), stop=(kb == kmax - 1))
                # evacuate PSUM immediately, then a per-(pair,Qi) batched
                # reciprocal: l rows DMA-reshaped [1,512]->[128,4] so the
                # DVE reciprocal (~8 cyc/elem PER LANE) sees 8 elems/lane
                yevs = []
                for h, yy in ((0, ya), (1, yb)):
                    yev = yevp.tile([65, 512], f32, tag="yev")
                    nc.vector.tensor_copy(yev[:, :], yy[0:65, :])
                    yevs.append(yev)
                stg = stgp.tile([128, 8], f32, tag="stg")
                rstg = stgp.tile([128, 8], f32, tag="rstg")
                for h in (0, 1):
                    nc.sync.dma_start(out=stg[:, h * 4:(h + 1) * 4],
                                      in_=yevs[h][64:65, :])
                nc.vector.reciprocal(rstg[:, :], stg[:, :])
                for h in (0, 1):
                    rr = smallp.tile([1, 512], f32, tag="rr")
                    nc.sync.dma_start(out=rr[0:1, :],
                                      in_=rstg[:, h * 4:(h + 1) * 4])
                    bc = bcp.tile([64, 512], f32, tag="bc")
                    nc.gpsimd.partition_broadcast(bc[:, :], rr[0:1, :])
                    nc.vector.tensor_mul(
                        yn[hp][h * 64:(h + 1) * 64, Qi * 512:(Qi + 1) * 512],
                        yevs[h][0:64, :], bc[:, :])

        def emit_z(Qi):
            for i in range(4 * Qi, 4 * Qi + 4):
                for j2 in range(2):
                    pz = ps.tile([128, 512], f32, tag="ps", name=f"pz{i}_{j2}")
                    for hp in range(NPAIR):
                        nc.tensor.matmul(
                            pz[:, :],
                            lhsT=yn[hp][:, i * 128:(i + 1) * 128],
                            rhs=wp_sb[hp][:, j2 * 512:(j2 + 1) * 512],
                            start=(hp == 0), stop=(hp == NPAIR - 1))
                    zs = zstp.tile([128, 512], mybir.dt.float16, tag="zst")
                    if (i * 2 + j2) % 2 == 0:
                        nc.vector.tensor_copy(zs[:, :], pz[:, :])
                    else:
                        nc.scalar.copy(zs[:, :], pz[:, :])
                    nc.sync.dma_start(
                        out=zD[i * 128:(i + 1) * 128,
                               j2 * 512:(j2 + 1) * 512],
                        in_=zs[:, :])

        # ---- schedule: pair 0's attention interleaves with the
        # v-projection so ScalarE's exp stream starts early; later pairs'
        # q/k projections are spread between the previous pair's attention
        # blocks (PE filler under the ACT-bound attention stretches);
        # pair 3 walks its query blocks in descending order with matching
        # z blocks right after, so the output projection chases pair 3.
        for j in range(NQB):
            emit_qkproj_part(0, j)
        emit_vproj(0, 4)
        emit_attention(0, 0)
        emit_vproj(4, 8)
        emit_attention(0, 1)
        emit_vproj(8, 12)
        emit_attention(0, 2)
        emit_vproj(12, 16)
        emit_qkproj_part(1, 0)
        emit_attention(0, 3)
        for j in range(1, NQB):
            emit_qkproj_part(1, j)
        for hp in (1, 2):
            for Qi in range(NQB):
                emit_attention(hp, Qi)
                emit_qkproj_part(hp + 1, Qi if hp < 2 else NQB - 1 - Qi)
        for Qi in range(NQB - 1, -1, -1):
            emit_attention(3, Qi)
            emit_z(Qi)

    nc.compile()
    return nc


def get_nc():
    global _CACHED_NC
    if _CACHED_NC is None:
        _CACHED_NC = _build_nc()
    return _CACHED_NC


def make_in_map(core, x, Wq, bq, Wk, bk, Wv, Wp):
    """Host-side shard/layout prep for one core (pure numpy, no FLOP-bearing
    compute: transposes, slicing, dtype casts)."""
    b, r = core // 2, core % 2
    hsl = slice(r * DH, (r + 1) * DH)
    return {
        "xT": np.ascontiguousarray(x[b].T).astype(BF),
        "wqT": np.ascontiguousarray(Wq[hsl, :].T).astype(BF),
        "wkT": np.ascontiguousarray(Wk[hsl, :].T).astype(BF),
        "wvT": np.ascontiguousarray(Wv[hsl, :].T).astype(BF),
        "wpT": np.ascontiguousarray(Wp[:, hsl].T).astype(BF),
        "bq": np.ascontiguousarray(bq[hsl]).reshape(NPAIR, 128, 1)
                .astype(np.float32),
        "bk": np.ascontiguousarray(bk[hsl]).reshape(NPAIR, 128, 1)
                .astype(np.float32),
        "tri": np.triu(np.ones((128, 128), np.float32)).astype(BF),
    }


def kernel(x, Wq, bq, Wk, bk, Wv, bv, Wp):
    global LAST_RESULTS
    x = np.asarray(x, np.float32)
    Wq, bq = np.asarray(Wq, np.float32), np.asarray(bq, np.float32)
    Wk, bk = np.asarray(Wk, np.float32), np.asarray(bk, np.float32)
    Wv, bv = np.asarray(Wv, np.float32), np.asarray(bv, np.float32)
    Wp = np.asarray(Wp, np.float32)

    nc = get_nc()
    in_maps = [make_in_map(c, x, Wq, bq, Wk, bk, Wv, Wp)
               for c in range(NCORES)]
    res = run_bass_kernel_spmd(nc, in_maps, core_ids=list(range(NCORES)))
    LAST_RESULTS = res

    # unshard: sum the two head-half partials per batch; add folded V-bias
    # term (y gets +bv per token; through the output projection that is the
    # constant vector Wp @ bv added to every token)
    zbias = (Wp @ bv).astype(np.float32)
    out = np.empty((B, T, C), np.float32)
    for b in range(B):
        za = np.asarray(res.results[2 * b]["z"], np.float32)
        zb = np.asarray(res.results[2 * b + 1]["z"], np.float32)
        out[b] = za + zb + zbias[None, :]
    return out
